# revision 5
# baseline (speedup 1.0000x reference)
"""Trainium2 Bass kernel (v8) for the attention-scoring module.

    q = query @ Wq.T + bq                               # (B, D)
    ref[b,d,k] = sum_e enc[k,b,e] * Wref[d,e] + bref[d]
    u[b,k] = sum_d v[d] * tanh(ref[b,d,k] + q[b,d])
    out = 10 * tanh(u)                                  # (B, K)

Data-parallel over batch: core c owns b in [32c, 32c+32).

v8 changes vs v7 (160.2us):
  - ALL rows on fp8 DoubleRow matmuls.  W quantization error removed via a
    residual stationary: W*S_W ~= fp8(W*S_W) + fp8(residual), both
    accumulated in PSUM at the same descale.  24 of 32 b-rows use
    enc-hi only (rel err contribution ~1.3e-5/row); 8 rows additionally
    carry an fp8 enc residual (3 matmul passes, ~bf16-exact).
    Simulated end-to-end rel err 1.781e-2 < 2e-2.
  - 512-wide DR chunks (half the matmul instruction count of v7's 256).
  - uniform activation scale (single DESCALE) for every row.
  - final logit emitted in 3 chunks (rows 0:64 / 64:96 / 96:128) so the
    tail after the last tanh is a short chain.
"""

import os
import sys

import numpy as np

os.environ.setdefault("JAX_COMPILATION_CACHE_DIR", "/tmp/jaxcache")

for _p in ("/opt/trn_rl_repo", "/opt/pypackages"):
    if _p not in sys.path:
        sys.path.append(_p)

import ml_dtypes

E = 256
D = 256
K = 2048
B = 256
NCORES = 8
BL = B // NCORES          # 32 batch rows per core
SLAB_B = 4                # b-rows per enc DMA slab
SLAB_N = SLAB_B * K       # 8192
# slab types: 'f' = fp8 hi-only (W-corrected), 'c' = fp8 hi+lo enc residual
TYPES = ("f", "f", "c", "f", "f", "c", "f", "f")
NF = TYPES.count("f") * SLAB_B        # 24 f-rows
NC_ = TYPES.count("c") * SLAB_B       # 8 c-rows
C_CLIP = 10.0
S_ENC = 16.0
S_W = 512.0
DESCALE = 1.0 / (S_ENC * S_W)

_compiled = None
last_exec_time_ns = None
last_results = None


def _build():
    from concourse import bacc, bass, tile
    from concourse.alu_op_type import AluOpType as ALU

    mybir = bass.mybir
    dt = mybir.dt
    AF = mybir.ActivationFunctionType

    nc = bacc.Bacc("TRN2", target_bir_lowering=False, debug=False,
                   num_devices=NCORES)

    # fp8 enc, rows 0:128 = e 0:128 (ec0), 128:256 = ec1; columns b-major
    encf_t = nc.declare_dram_parameter("encf", [E, NF * K], dt.float8e4, isOutput=False)
    ench_t = nc.declare_dram_parameter("ench", [E, NC_ * K], dt.float8e4, isOutput=False)
    encl_t = nc.declare_dram_parameter("encl", [E, NC_ * K], dt.float8e4, isOutput=False)
    cf32_t = nc.declare_dram_parameter("cf32", [128, 581], dt.float32, isOutput=False)
    cfp8_t = nc.declare_dram_parameter("cfp8", [128, 1024], dt.float8e4, isOutput=False)
    cfp16_t = nc.declare_dram_parameter("cfp16", [128, 1], dt.float16, isOutput=False)
    out_p = nc.declare_dram_parameter("out", [128, 512], dt.float32, isOutput=True)

    with tile.TileContext(nc) as tc:
        with (
            tc.tile_pool(name="const", bufs=1) as constp,
            tc.tile_pool(name="enc", bufs=2) as encp,
            tc.tile_pool(name="tt", bufs=10) as tp,
            tc.tile_pool(name="tail", bufs=2) as tailp,
            tc.tile_pool(name="psum_m", bufs=3, space="PSUM") as pmp,
            tc.tile_pool(name="psum_s", bufs=2, space="PSUM") as psp,
        ):
            # ---- constants: packed DMAs ----
            cfp8_sb = constp.tile([128, 1024], dt.float8e4)
            cf32_sb = constp.tile([128, 581], dt.float32)
            cfp16_sb = constp.tile([128, 1], dt.float16)
            bias_sb = constp.tile([128, 2 * BL], dt.float32)   # [:, dc*32 + b]
            u_sb = constp.tile([128, 512], dt.float32)         # [b*4+jj, kk]
            nc.sync.dma_start(cfp8_sb[:], cfp8_t[:])
            nc.sync.dma_start(cf32_sb[:], cf32_t[:])
            nc.sync.dma_start(cfp16_sb[:], cfp16_t[:])

            wq_sb = cf32_sb[:, 0:512]        # [:, (ec*2+dc)*128 + d]
            query_sb = cf32_sb[:, 512:576]   # [:, ec*32 + b]
            cbias_sb = cf32_sb[:, 576:578]
            ratio_sb = cf32_sb[:, 578:579]   # v1/v0 per pair-partition
            whi_sb = cfp8_sb[:, 0:512]       # fp8 W-hi pairs [dc*256 + i*128 + m]
            wlo_sb = cfp8_sb[:, 512:1024]    # fp8 W residual, same layout
            v0_sb = cfp16_sb[:, 0:1]         # fp16 |v| even-rank column

            # ---- enc slab loading ----
            def alloc_slab(typ, s):
                if typ == "f":
                    return (encp.tile([128, 2 * SLAB_N], dt.float8e4, tag="encf",
                                      name=f"encf_s{s}"),)
                return (encp.tile([128, 2 * SLAB_N], dt.float8e4, tag="ench",
                                  name=f"ench_s{s}"),
                        encp.tile([128, 2 * SLAB_N], dt.float8e4, tag="encl",
                                  name=f"encl_s{s}"))

            def emit_pieces(typ, tiles, j, q0, q1, dep=None):
                # piece q covers b_in q of the slab (2048 columns)
                w = K
                srcs = (encf_t,) if typ == "f" else (ench_t, encl_t)
                for q in range(q0, q1):
                    for t, src in zip(tiles, srcs):
                        for ec in range(2):
                            ins = nc.sync.dma_start(
                                t[:, ec * SLAB_N + q * w:ec * SLAB_N + (q + 1) * w],
                                src[ec * 128:(ec + 1) * 128,
                                    j * SLAB_N + q * w:j * SLAB_N + (q + 1) * w])
                            if dep is not None:
                                tile.add_dep_helper(ins.ins, dep.ins,
                                                    reason="defer enc prefetch")

            # per-type dram slab index for each slab seq position
            f_idx, c_idx, slab_j = 0, 0, []
            for t in TYPES:
                if t == "f":
                    slab_j.append(f_idx); f_idx += 1
                else:
                    slab_j.append(c_idx); c_idx += 1

            slab0 = alloc_slab(TYPES[0], 0)
            emit_pieces(TYPES[0], slab0, slab_j[0], 0, 1)

            # ---- q_rawT = (query @ Wq'.T).T per dc-chunk, + (bref' + bq') ----
            for dc in range(2):
                qps = psp.tile([128, BL], dt.float32, tag="st")
                for ec in range(2):
                    nc.tensor.matmul(
                        qps[:],
                        wq_sb[:, (ec * 2 + dc) * 128:(ec * 2 + dc + 1) * 128],
                        query_sb[:, ec * BL:(ec + 1) * BL],
                        start=(ec == 0), stop=(ec == 1),
                    )
                nc.vector.tensor_scalar_add(bias_sb[:, dc * BL:(dc + 1) * BL],
                                            qps[:], cbias_sb[:, dc:dc + 1])

            # ---- per-b tail: combine over both kp halves at once ----
            def emit_w(tts, b):
                # w' = t0 + ratio * t1  (TS 2x + TT 2x), fp16, 2048 wide
                w1 = tp.tile([128, 2048], dt.float16, tag="w1", bufs=2)
                nc.vector.tensor_scalar_mul(w1[:], tts[1][:], ratio_sb[:, 0:1])
                w = tp.tile([128, 2048], dt.float16, tag="w", bufs=2)
                nc.vector.tensor_add(w[:], w1[:], tts[0][:])
                return w

            def emit_strips(st4, w, b):
                for jj in range(4):
                    nc.tensor.matmul(
                        st4[32 * jj:32 * jj + 1, :],
                        v0_sb,
                        w[:, jj * 512:(jj + 1) * 512],
                        start=True, stop=True,
                        skip_group_check=True,
                        tile_position=(0, 32 * jj),
                    )
                sp = tailp.tile([128, 512], dt.float32, tag="sp")
                nc.vector.tensor_copy(sp[:], st4[:])
                nc.gpsimd.dma_start(u_sb[4 * b:4 * b + 4, :],
                                    sp[0:128:32, :])

            t6 = constp.tile([128, 512], dt.float32)
            o6 = constp.tile([128, 512], dt.float32)

            def emit_final(rows):
                nc.scalar.activation(t6[rows, :], u_sb[rows, :], AF.Tanh)
                nc.vector.tensor_scalar_mul(o6[rows, :], t6[rows, :], C_CLIP)
                nc.sync.dma_start(out_p[rows, :], o6[rows, :])

            pend = []
            prev_mm = None

            def tail_pump():
                while len(pend) > 1:
                    emit_strips(*pend.pop(0))

            # ---- unified main loop over slabs ----
            def pair(t):
                return t[:].rearrange("p (two n) -> p two n", two=2)

            cur_slab = slab0
            for si, typ in enumerate(TYPES):
                nxt_slab = None
                for b_in in range(SLAB_B):
                    b = SLAB_B * si + b_in
                    st4 = psp.tile([128, 512], dt.float32, tag="st")
                    btts = [tp.tile([128, 2048], dt.float16, tag="tt", bufs=5,
                                    name=f"tt_{b}_{dcx}")
                            for dcx in range(2)]
                    hi_pair = pair(cur_slab[0])
                    lo_pair = pair(cur_slab[1]) if typ == "c" else None
                    for kp in range(2):
                        if si == 0 and (b_in, kp) == (0, 1):
                            emit_pieces(typ, cur_slab, slab_j[0], 1, 2, dep=prev_mm)
                        if si == 0 and (b_in, kp) == (1, 0):
                            emit_pieces(typ, cur_slab, slab_j[0], 2, 4, dep=prev_mm)
                        pf_now = ((b_in, kp) == (2, 0)) if si == 0 else \
                                 ((b_in, kp) == (0, 1))
                        if pf_now and si + 1 < len(TYPES):
                            nxt_slab = alloc_slab(TYPES[si + 1], si + 1)
                            emit_pieces(TYPES[si + 1], nxt_slab, slab_j[si + 1],
                                        0, 4, dep=prev_mm)
                        first_mm = None
                        for dc in range(2):
                            psd = pmp.tile([128, 1024], dt.float32, tag="psd")
                            whi = whi_sb[:, dc * 256:(dc + 1) * 256].rearrange(
                                "p (two m) -> p two m", two=2)
                            wlo = wlo_sb[:, dc * 256:(dc + 1) * 256].rearrange(
                                "p (two m) -> p two m", two=2)
                            # pass list: (stationary, moving, start, stop)
                            if typ == "f":
                                passes = [(whi, hi_pair, True, False),
                                          (wlo, hi_pair, False, True)]
                            else:
                                passes = [(whi, hi_pair, True, False),
                                          (whi, lo_pair, False, False),
                                          (wlo, hi_pair, False, True)]
                            for stat, mov, st_f, sp_f in passes:
                                for kb in range(2):
                                    nseg = b_in * K + kp * 1024 + kb * 512
                                    ins = nc.tensor.matmul(
                                        psd[:, kb * 512:(kb + 1) * 512],
                                        stat,
                                        mov[:, :, nseg:nseg + 512],
                                        start=st_f, stop=sp_f,
                                        perf_mode=mybir.MatmulPerfMode.DoubleRow,
                                        skip_group_check=True,
                                    )
                                    if first_mm is None:
                                        first_mm = ins
                            nc.scalar.activation(
                                btts[dc][:, kp * 1024:(kp + 1) * 1024],
                                psd[:], AF.Tanh,
                                bias=bias_sb[:, dc * BL + b:dc * BL + b + 1],
                                scale=DESCALE)
                        if kp == 1:
                            w = emit_w(btts, b)
                            pend.append((st4, w, b))
                            tail_pump()
                        prev_mm = first_mm
                        # finals once the needed strips (pumped with 1-b lag)
                        # have been emitted: b15 strips emit during b16 kp=1,
                        # b23 strips during b24 kp=1
                        if (si, b_in, kp) == (4, 1, 0):
                            emit_final(slice(0, 64))
                        if (si, b_in, kp) == (6, 1, 0):
                            emit_final(slice(64, 96))
                cur_slab = nxt_slab
            for args in pend:
                emit_strips(*args)
            emit_final(slice(96, 128))

    nc.compile()
    return nc


def _prep_inputs(encoder_output, query, Wq, bq, Wref, bref, v):
    fp16 = np.float16
    e4 = ml_dtypes.float8_e4m3fn if hasattr(ml_dtypes, "float8_e4m3fn") else ml_dtypes.float8_e4m3

    v = np.asarray(v, np.float32)
    sgn = np.where(v >= 0, 1.0, -1.0).astype(np.float32)
    va = np.abs(v)
    order = np.argsort(va, kind="stable")
    c0_idx, c1_idx = order[0::2], order[1::2]
    v0, v1 = va[c0_idx], va[c1_idx]
    ratio = (v1 / v0).astype(np.float32)
    perm = np.concatenate([c0_idx, c1_idx])          # new d order (dc-major)

    Wp = (np.asarray(Wref, np.float32) * sgn[:, None])[perm]     # (256, 256)
    Wqp = (np.asarray(Wq, np.float32) * sgn[:, None])[perm]
    cbias = (np.asarray(bref, np.float32) + np.asarray(bq, np.float32)) * sgn
    cbias = cbias[perm]

    def chunk4(w):                                   # (E, 256d) -> (512, 128)
        return np.ascontiguousarray(
            w.reshape(2, 128, 2, 128).transpose(0, 2, 1, 3).reshape(512, 128))

    def pack(w4):                                    # (4*128, X) -> (128, 4*X)
        x = w4.shape[1]
        return w4.reshape(4, 128, x).transpose(1, 0, 2).reshape(128, 4 * x)

    wq_p = pack(chunk4(np.ascontiguousarray(Wqp.T))) # (128, 512) f32

    # fp8 W pairs: per dc, lhsT[e, i, m] = W[dc*128+m, i*128+e] * S_W
    def w_pack(Wf32):
        w8 = np.empty((128, 512), np.float32)
        for dc in range(2):
            chunk = Wf32[dc * 128:(dc + 1) * 128]    # (128 d, 256 e)
            for i in range(2):
                w8[:, dc * 256 + i * 128:dc * 256 + (i + 1) * 128] = \
                    chunk[:, i * 128:(i + 1) * 128].T
        return w8

    # residual computed in the packed domain (packing is a permutation)
    WpS_packed = w_pack(Wp * S_W)
    whi = np.clip(WpS_packed, -240.0, 240.0).astype(e4)
    wlo = np.clip(WpS_packed - whi.astype(np.float32),
                  -240.0, 240.0).astype(e4)

    cbias_p = cbias.reshape(2, 128).T                # (128, 2)
    ratio_p = ratio.reshape(128, 1)
    v0_p = v0.reshape(128, 1).astype(fp16)
    queryT = np.ascontiguousarray(np.asarray(query, np.float32).T)  # (E, B)

    # c-rows within each core's 32-row block
    c_rows = []
    f_rows = []
    for s, t in enumerate(TYPES):
        (c_rows if t == "c" else f_rows).extend(range(4 * s, 4 * s + 4))

    enc = np.asarray(encoder_output, np.float32)     # (K, B, E)
    encT = enc.transpose(2, 1, 0)                    # (E, B, K) view

    in_maps = []
    for c in range(NCORES):
        bs = slice(c * BL, (c + 1) * BL)
        enc_c = encT[:, bs, :]                       # (E, 32, K)
        encf_rows = np.ascontiguousarray(enc_c[:, f_rows, :]).reshape(E, NF * K)
        encf = np.clip(encf_rows * S_ENC, -240.0, 240.0).astype(e4)
        encc_rows = np.ascontiguousarray(enc_c[:, c_rows, :]).reshape(E, NC_ * K)
        encc_s = encc_rows * S_ENC
        ench = np.clip(encc_s, -240.0, 240.0).astype(e4)
        encl = np.clip(encc_s - ench.astype(np.float32),
                       -240.0, 240.0).astype(e4)

        q_c = queryT[:, bs]                          # (256, 32)
        q_p = q_c.reshape(2, 128, BL).transpose(1, 0, 2).reshape(128, 2 * BL)
        cf32 = np.ascontiguousarray(np.concatenate(
            [wq_p, q_p, cbias_p, ratio_p,
             np.zeros((128, 2), np.float32)], axis=1), dtype=np.float32)
        cfp8 = np.concatenate([np.asarray(whi), np.asarray(wlo)], axis=1)
        in_maps.append({
            "encf": encf,
            "ench": ench,
            "encl": encl,
            "cf32": cf32,
            "cfp8": np.ascontiguousarray(cfp8),
            "cfp16": v0_p,
        })
    return in_maps


def kernel(**inputs):
    global _compiled, last_exec_time_ns, last_results
    from concourse import bass_utils

    if _compiled is None:
        _compiled = _build()
    nc = _compiled

    in_maps = _prep_inputs(**inputs)
    res = bass_utils.run_bass_kernel_spmd(nc, in_maps, core_ids=list(range(NCORES)))
    last_exec_time_ns = res.exec_time_ns
    last_results = res
    out = np.concatenate(
        [r["out"].reshape(BL, K) for r in res.results], axis=0)
    return out


# revision 6
# speedup vs baseline: 1.1087x; 1.1087x over previous
"""Trainium2 Bass kernel (v9) for the attention-scoring module.

    q = query @ Wq.T + bq                               # (B, D)
    ref[b,d,k] = sum_e enc[k,b,e] * Wref[d,e] + bref[d]
    u[b,k] = sum_d v[d] * tanh(ref[b,d,k] + q[b,d])
    out = 10 * tanh(u)                                  # (B, K)

Data-parallel over batch: core c owns b in [32c, 32c+32).

v9 (from v7 @160.2us; v8's multi-pass fp8 regressed: PE runs at 1 col/cycle
regardless of dtype -- DoubleRow only doubles contraction per instruction):
  - 14 of 32 rows on single-pass fp8 DR (sim rel err 1.80e-2), slabs of 2
    b-rows so the 14/18 split aligns to slab granularity.
  - fp8 DR chunks 512 wide (half the instruction count of v7's 256).
  - final logit emitted in 3 chunks (after b15 / b23 / end) to shorten the
    serial tail after the last tanh.
  - constants DMA reordered (w8 + first enc piece before the big cf32) to
    cut scalar startup latency.
"""

import os
import sys

import numpy as np

os.environ.setdefault("JAX_COMPILATION_CACHE_DIR", "/tmp/jaxcache")

for _p in ("/opt/trn_rl_repo", "/opt/pypackages"):
    if _p not in sys.path:
        sys.path.append(_p)

import ml_dtypes

E = 256
D = 256
K = 2048
B = 256
NCORES = 8
BL = B // NCORES          # 32 batch rows per core
SLAB_B = 2                # b-rows per enc DMA slab
SLAB_N = SLAB_B * K       # 4096
NSLAB = BL // SLAB_B      # 16
# slab type sequence: '8' = fp8 single-pass, 'b' = bf16; 7*2=14 fp8 rows
SLAB_SEQ = ("8", "b", "8", "b", "8", "b", "8", "b",
            "8", "b", "8", "b", "8", "b", "b", "b")
NB8 = SLAB_SEQ.count("8") * SLAB_B     # 14
NBB = BL - NB8                          # 18
N8 = NB8 * K
NB = NBB * K
C_CLIP = 10.0
S_ENC = 16.0
S_W = 512.0
DESCALE = 1.0 / (S_ENC * S_W)

_compiled = None
last_exec_time_ns = None
last_results = None


def _build():
    from concourse import bacc, bass, tile
    from concourse.alu_op_type import AluOpType as ALU

    mybir = bass.mybir
    dt = mybir.dt
    AF = mybir.ActivationFunctionType

    nc = bacc.Bacc("TRN2", target_bir_lowering=False, debug=False,
                   num_devices=NCORES)

    # fp8 enc: rows 0:128 = stream A (e 0:128), 128:256 = B
    enc8_t = nc.declare_dram_parameter("enc8", [E, N8], dt.float8e4, isOutput=False)
    # bf16 enc, two row-halves
    encb_t = nc.declare_dram_parameter("encb", [E, NB], dt.bfloat16, isOutput=False)
    cf32_t = nc.declare_dram_parameter("cf32", [128, 581], dt.float32, isOutput=False)
    cbf16_t = nc.declare_dram_parameter("cbf16", [128, 512], dt.bfloat16, isOutput=False)
    cfp8_t = nc.declare_dram_parameter("cfp8", [128, 512], dt.float8e4, isOutput=False)
    cfp16_t = nc.declare_dram_parameter("cfp16", [128, 1], dt.float16, isOutput=False)
    out_p = nc.declare_dram_parameter("out", [128, 512], dt.float32, isOutput=True)

    with tile.TileContext(nc) as tc:
        with (
            tc.tile_pool(name="const", bufs=1) as constp,
            tc.tile_pool(name="enc", bufs=3) as encp,
            tc.tile_pool(name="tt", bufs=10) as tp,
            tc.tile_pool(name="tail", bufs=2) as tailp,
            tc.tile_pool(name="psum_m", bufs=3, space="PSUM") as pmp,
            tc.tile_pool(name="psum_s", bufs=2, space="PSUM") as psp,
        ):
            # ---- constants: packed DMAs (small/urgent first) ----
            cfp8_sb = constp.tile([128, 512], dt.float8e4)
            cf32_sb = constp.tile([128, 581], dt.float32)
            cbf16_sb = constp.tile([128, 512], dt.bfloat16)
            cfp16_sb = constp.tile([128, 1], dt.float16)
            bias_sb = constp.tile([128, 2 * BL], dt.float32)   # [:, dc*32 + b]
            u_sb = constp.tile([128, 512], dt.float32)         # [b*4+jj, kk]
            nc.sync.dma_start(cfp8_sb[:], cfp8_t[:])

            # ---- enc slab loading ----
            def alloc_slab8(s):
                return encp.tile([128, 2 * SLAB_N], dt.float8e4, tag="enc8",
                                 name=f"enc8_s{s}")

            def emit_pieces8(t8, s, q0, q1, dep=None):
                w = K
                for q in range(q0, q1):
                    for ec in range(2):
                        ins = nc.sync.dma_start(
                            t8[:, ec * SLAB_N + q * w:ec * SLAB_N + (q + 1) * w],
                            enc8_t[ec * 128:(ec + 1) * 128,
                                   s * SLAB_N + q * w:s * SLAB_N + (q + 1) * w])
                        if dep is not None:
                            tile.add_dep_helper(ins.ins, dep.ins,
                                                reason="defer enc prefetch")

            def alloc_slabb(s):
                return [encp.tile([128, SLAB_N], dt.bfloat16, tag=f"encb{ec}",
                                  name=f"encb{ec}_s{s}")
                        for ec in range(2)]

            def emit_piecesb(tiles, s, q0, q1, dep=None):
                w = K
                for q in range(q0, q1):
                    for ec in range(2):
                        ins = nc.sync.dma_start(
                            tiles[ec][:, q * w:(q + 1) * w],
                            encb_t[ec * 128:(ec + 1) * 128,
                                   s * SLAB_N + q * w:s * SLAB_N + (q + 1) * w])
                        if dep is not None:
                            tile.add_dep_helper(ins.ins, dep.ins,
                                                reason="defer enc prefetch")

            # per-type dram slab index per seq position
            i8, ib, slab_j = 0, 0, []
            for t in SLAB_SEQ:
                if t == "8":
                    slab_j.append(i8); i8 += 1
                else:
                    slab_j.append(ib); ib += 1

            slab0 = alloc_slab8(slab_j[0])
            emit_pieces8(slab0, slab_j[0], 0, 1)

            nc.sync.dma_start(cf32_sb[:], cf32_t[:])
            nc.sync.dma_start(cbf16_sb[:], cbf16_t[:])
            nc.sync.dma_start(cfp16_sb[:], cfp16_t[:])

            wq_sb = cf32_sb[:, 0:512]        # [:, (ec*2+dc)*128 + d]
            query_sb = cf32_sb[:, 512:576]   # [:, ec*32 + b]
            cbias_sb = cf32_sb[:, 576:578]
            ratio_sb = cf32_sb[:, 578:579]   # v1/v0 per pair-partition
            wref_sb = cbf16_sb[:, 0:512]     # bf16 W [(ec*2+dc)*128 + d]
            w8_sb = cfp8_sb[:, 0:512]        # fp8 W pairs [dc*256 + i*128 + m]
            v0_sb = cfp16_sb[:, 0:1]         # fp16 |v| even-rank column

            # ---- q_rawT = (query @ Wq'.T).T per dc-chunk, + (bref' + bq') ----
            for dc in range(2):
                qps = psp.tile([128, BL], dt.float32, tag="st")
                for ec in range(2):
                    nc.tensor.matmul(
                        qps[:],
                        wq_sb[:, (ec * 2 + dc) * 128:(ec * 2 + dc + 1) * 128],
                        query_sb[:, ec * BL:(ec + 1) * BL],
                        start=(ec == 0), stop=(ec == 1),
                    )
                nc.vector.tensor_scalar_add(bias_sb[:, dc * BL:(dc + 1) * BL],
                                            qps[:], cbias_sb[:, dc:dc + 1])

            # ---- per-b tail: combine over both kp halves at once ----
            def emit_w(tts, b):
                # w' = t0 + ratio * t1  (TS 2x + TT 2x), fp16, 2048 wide
                w1 = tp.tile([128, 2048], dt.float16, tag="w1", bufs=2)
                nc.vector.tensor_scalar_mul(w1[:], tts[1][:], ratio_sb[:, 0:1])
                w = tp.tile([128, 2048], dt.float16, tag="w", bufs=2)
                nc.vector.tensor_add(w[:], w1[:], tts[0][:])
                return w

            def emit_strips(st4, w, b):
                for jj in range(4):
                    nc.tensor.matmul(
                        st4[32 * jj:32 * jj + 1, :],
                        v0_sb,
                        w[:, jj * 512:(jj + 1) * 512],
                        start=True, stop=True,
                        skip_group_check=True,
                        tile_position=(0, 32 * jj),
                    )
                sp = tailp.tile([128, 512], dt.float32, tag="sp")
                nc.vector.tensor_copy(sp[:], st4[:])
                nc.gpsimd.dma_start(u_sb[4 * b:4 * b + 4, :],
                                    sp[0:128:32, :])

            t6 = constp.tile([128, 512], dt.float32)
            o6 = constp.tile([128, 512], dt.float32)

            def emit_final(rows):
                nc.scalar.activation(t6[rows, :], u_sb[rows, :], AF.Tanh)
                nc.vector.tensor_scalar_mul(o6[rows, :], t6[rows, :], C_CLIP)
                nc.sync.dma_start(out_p[rows, :], o6[rows, :])

            pend = []
            prev_mm = None

            def tail_pump():
                while len(pend) > 1:
                    emit_strips(*pend.pop(0))

            # ---- unified main loop: fp8 / bf16 slabs interleaved ----
            def alloc_and_prefetch(si, dep):
                typ, j = SLAB_SEQ[si], slab_j[si]
                if typ == "8":
                    t = alloc_slab8(j)
                    emit_pieces8(t, j, 0, SLAB_B, dep=dep)
                else:
                    t = alloc_slabb(j)
                    emit_piecesb(t, j, 0, SLAB_B, dep=dep)
                return t

            cur_slab = slab0
            for si, typ in enumerate(SLAB_SEQ):
                nxt_slab = None
                for b_in in range(SLAB_B):
                    b = SLAB_B * si + b_in
                    st4 = psp.tile([128, 512], dt.float32, tag="st")
                    btts = [tp.tile([128, 2048], dt.float16, tag="tt", bufs=5,
                                    name=f"tt_{b}_{dcx}")
                            for dcx in range(2)]
                    if typ == "8":
                        enc_pair = cur_slab[:].rearrange("p (two n) -> p two n", two=2)
                    for kp in range(2):
                        if si == 0 and (b_in, kp) == (0, 1):
                            emit_pieces8(cur_slab, slab_j[0], 1, 2, dep=prev_mm)
                        pf_now = ((b_in, kp) == (1, 0)) if si == 0 else \
                                 ((b_in, kp) == (0, 1))
                        if pf_now and si + 1 < NSLAB:
                            nxt_slab = alloc_and_prefetch(si + 1, prev_mm)
                        first_mm = None
                        for dc in range(2):
                            psd = pmp.tile([128, 1024], dt.float32, tag="psd")
                            if typ == "8":
                                for kb in range(2):
                                    nseg = b_in * K + kp * 1024 + kb * 512
                                    ins = nc.tensor.matmul(
                                        psd[:, kb * 512:(kb + 1) * 512],
                                        w8_sb[:, dc * 256:(dc + 1) * 256].rearrange(
                                            "p (two m) -> p two m", two=2),
                                        enc_pair[:, :, nseg:nseg + 512],
                                        start=True, stop=True,
                                        perf_mode=mybir.MatmulPerfMode.DoubleRow,
                                        skip_group_check=True,
                                    )
                                    if first_mm is None:
                                        first_mm = ins
                            else:
                                for ec in range(2):
                                    for kb in range(2):
                                        nseg = b_in * K + kp * 1024 + kb * 512
                                        ins = nc.tensor.matmul(
                                            psd[:, kb * 512:(kb + 1) * 512],
                                            wref_sb[:, (ec * 2 + dc) * 128:(ec * 2 + dc + 1) * 128],
                                            cur_slab[ec][:, nseg:nseg + 512],
                                            start=(ec == 0), stop=(ec == 1),
                                            skip_group_check=True,
                                        )
                                        if first_mm is None:
                                            first_mm = ins
                            nc.scalar.activation(
                                btts[dc][:, kp * 1024:(kp + 1) * 1024],
                                psd[:], AF.Tanh,
                                bias=bias_sb[:, dc * BL + b:dc * BL + b + 1],
                                scale=(DESCALE if typ == "8" else 1.0))
                        if kp == 1:
                            w = emit_w(btts, b)
                            pend.append((st4, w, b))
                            tail_pump()
                        prev_mm = first_mm
                        # b15 strips emitted once b16 is pushed (si=8, b_in=0,
                        # kp=1); b23's once b24 is pushed (si=12, b_in=0, kp=1)
                        if (si, b_in, kp) == (8, 1, 0):
                            emit_final(slice(0, 64))
                        if (si, b_in, kp) == (12, 1, 0):
                            emit_final(slice(64, 96))
                cur_slab = nxt_slab
            for args in pend:
                emit_strips(*args)
            emit_final(slice(96, 128))

    nc.compile()
    return nc


def _prep_inputs(encoder_output, query, Wq, bq, Wref, bref, v):
    bf16 = ml_dtypes.bfloat16
    fp16 = np.float16
    e4 = ml_dtypes.float8_e4m3fn if hasattr(ml_dtypes, "float8_e4m3fn") else ml_dtypes.float8_e4m3

    v = np.asarray(v, np.float32)
    sgn = np.where(v >= 0, 1.0, -1.0).astype(np.float32)
    va = np.abs(v)
    order = np.argsort(va, kind="stable")
    c0_idx, c1_idx = order[0::2], order[1::2]
    v0, v1 = va[c0_idx], va[c1_idx]
    ratio = (v1 / v0).astype(np.float32)
    perm = np.concatenate([c0_idx, c1_idx])          # new d order (dc-major)

    Wp = (np.asarray(Wref, np.float32) * sgn[:, None])[perm]     # (256, 256)
    Wqp = (np.asarray(Wq, np.float32) * sgn[:, None])[perm]
    cbias = (np.asarray(bref, np.float32) + np.asarray(bq, np.float32)) * sgn
    cbias = cbias[perm]

    def chunk4(w):                                   # (E, 256d) -> (512, 128)
        return np.ascontiguousarray(
            w.reshape(2, 128, 2, 128).transpose(0, 2, 1, 3).reshape(512, 128))

    def pack(w4):                                    # (4*128, X) -> (128, 4*X)
        x = w4.shape[1]
        return w4.reshape(4, 128, x).transpose(1, 0, 2).reshape(128, 4 * x)

    # bf16 W pack: WT (E, D') where D' columns are [c0 | c1]
    WT = np.ascontiguousarray(Wp.T)                  # (E, 256) cols dc-major
    wref_p = pack(chunk4(WT)).astype(bf16)           # (128, 512)
    wq_p = pack(chunk4(np.ascontiguousarray(Wqp.T))) # (128, 512) f32

    # fp8 W pairs: per dc, lhsT[e, i, m] = Wp[dc*128+m, i*128+e] * S_W
    w8 = np.empty((128, 512), np.float32)
    for dc in range(2):
        chunk = Wp[dc * 128:(dc + 1) * 128] * S_W    # (128 d, 256 e)
        for i in range(2):
            w8[:, dc * 256 + i * 128:dc * 256 + (i + 1) * 128] = \
                chunk[:, i * 128:(i + 1) * 128].T
    w8 = np.clip(w8, -240.0, 240.0).astype(e4)

    cbias_p = cbias.reshape(2, 128).T                # (128, 2)
    ratio_p = ratio.reshape(128, 1)
    v0_p = v0.reshape(128, 1).astype(fp16)
    queryT = np.ascontiguousarray(np.asarray(query, np.float32).T)  # (E, B)

    # row lists by slab type (within each core's 32-row block)
    rows8, rowsb = [], []
    for s, t in enumerate(SLAB_SEQ):
        (rows8 if t == "8" else rowsb).extend(range(SLAB_B * s, SLAB_B * (s + 1)))

    enc = np.asarray(encoder_output, np.float32)     # (K, B, E)
    encT = enc.transpose(2, 1, 0)                    # (E, B, K) view

    in_maps = []
    for c in range(NCORES):
        bs = slice(c * BL, (c + 1) * BL)
        enc_c = encT[:, bs, :]                       # (E, 32, K)
        enc8 = np.ascontiguousarray(enc_c[:, rows8, :]).reshape(E, N8)
        enc8 = np.clip(enc8 * S_ENC, -240.0, 240.0).astype(e4)
        encb = np.ascontiguousarray(enc_c[:, rowsb, :]).reshape(E, NB).astype(bf16)

        q_c = queryT[:, bs]                          # (256, 32)
        q_p = q_c.reshape(2, 128, BL).transpose(1, 0, 2).reshape(128, 2 * BL)
        cf32 = np.ascontiguousarray(np.concatenate(
            [wq_p, q_p, cbias_p, ratio_p,
             np.zeros((128, 2), np.float32)], axis=1), dtype=np.float32)
        in_maps.append({
            "enc8": enc8,
            "encb": encb,
            "cf32": cf32,
            "cbf16": wref_p,
            "cfp8": w8,
            "cfp16": v0_p,
        })
    return in_maps


def kernel(**inputs):
    global _compiled, last_exec_time_ns, last_results
    from concourse import bass_utils

    if _compiled is None:
        _compiled = _build()
    nc = _compiled

    in_maps = _prep_inputs(**inputs)
    res = bass_utils.run_bass_kernel_spmd(nc, in_maps, core_ids=list(range(NCORES)))
    last_exec_time_ns = res.exec_time_ns
    last_results = res
    out = np.concatenate(
        [r["out"].reshape(BL, K) for r in res.results], axis=0)
    return out
